# revision 7
# baseline (speedup 1.0000x reference)
"""Trainium2 Bass kernel for nn_AttentionModule (channel self-attention).

Reference computation (per batch sample b, with x: [C=512, N=4096]):
    q   = w1 @ x + b1                     # [64, 4096]
    att = softmax(q @ q.T, axis=-1)       # [64, 64]
    out = att @ q                         # [64, 4096]
    y   = w2 @ out + b2 + x               # [512, 4096]

Sharding: data-parallel over batch. B=16 samples, 8 cores, 2 samples/core.
Small weights (w1,b1,w2,b2) replicated to every core.

v2 design (vs the f32r baseline at ~146us):
  - bf16 everywhere: x is converted to bf16 on the host and loaded as bf16;
    the output is stored bf16 and upcast on the host.  HBM traffic halves
    (33.6MB -> 16.8MB per core; wire roofline ~47us at ~360GB/s).
  - q transposes for the Gram go through the DMA XBAR transpose
    (dma_start_transpose, 16x128 tiles) instead of 64 PE transposes + 64
    scalar copies per core.  The Gram is layout-invariant to the XBAR's
    (t p) vs (p t) grouping since it sums over all n.
  - att@q and the w2 conv are fused: G^T = att^T @ w2T (one 512-row matmul)
    and y = Ga^T.T @ qa with K=65 (row 64 of qa is ones, row 64 of GaT is
    b2, so the bias rides the contraction).
  - residual + evacuation split: 10/16 of the [128,1024] PSUM units are
    evacuated by DVE tensor_add(+x), 6/16 get x pre-accumulated on the PE
    (identity matmul) and a plain scalar-engine copy, balancing DVE/ACT.
  - all x loads dispatch upfront on the sync HWDGE queue; stores follow
    FIFO on the same queue; DMA transposes ride the scalar HWDGE queue.
"""

import os
import sys
from contextlib import ExitStack

import numpy as np

for _p in ("/opt/trn_rl_repo", "/root/.axon_site/_ro/trn_rl_repo"):
    if os.path.isdir(_p) and _p not in sys.path:
        sys.path.append(_p)

import ml_dtypes  # noqa: E402

import concourse.bass as bass  # noqa: E402
import concourse.tile as tile  # noqa: E402
from concourse import bacc, mybir  # noqa: E402
from concourse.bass_utils import run_bass_kernel_spmd  # noqa: E402
from concourse.masks import make_identity  # noqa: E402

F32 = mybir.dt.float32
BF16 = mybir.dt.bfloat16
AF = mybir.ActivationFunctionType
ALU = mybir.AluOpType
AX = mybir.AxisListType

B, C, CR = 16, 512, 64
W, H = 64, 64
N = W * H  # 4096
NCORES = 8
BPC = B // NCORES  # samples per core
KC = C // 128  # 4 k-chunks of x / oc-chunks of output
NF = 512  # PSUM-bank moving width
NN = N // NF  # 8 n-blocks per sample
NT = N // 128  # 32 gram chunks
LF = 2048  # x load piece width (bf16 elements)
NL = N // LF  # 2 pieces per k-chunk row
EU = 1024  # y evacuation unit width
NU = N // EU  # 4 units per oc-chunk -> 16 per sample
# which of the 16 (oc, u) y-units per sample are evacuated by ACT
# (x added via PE identity-matmul); the rest go to DVE tensor_add.
ACT_UNITS = frozenset({1, 4, 7, 9, 12, 15})


def _build_nc():
    nc = bacc.Bacc(
        "TRN2",
        target_bir_lowering=False,
        debug=False,
        enable_asserts=True,
        num_devices=NCORES,
    )
    x_d = nc.dram_tensor("x", [BPC, C, N], BF16, kind="ExternalInput").ap()
    w1_d = nc.dram_tensor("w1", [CR, C], F32, kind="ExternalInput").ap()
    b1_d = nc.dram_tensor("b1", [CR], F32, kind="ExternalInput").ap()
    w2_d = nc.dram_tensor("w2", [C, CR], F32, kind="ExternalInput").ap()
    b2_d = nc.dram_tensor("b2", [C], F32, kind="ExternalInput").ap()
    out_d = nc.dram_tensor("out", [BPC, C, N], BF16, kind="ExternalOutput").ap()

    with tile.TileContext(nc) as tc, ExitStack() as ctx:
        singles = ctx.enter_context(tc.tile_pool(name="singles", bufs=1))
        fin = ctx.enter_context(tc.tile_pool(name="fin", bufs=8))
        small = ctx.enter_context(tc.tile_pool(name="small", bufs=2))
        ps_q = ctx.enter_context(tc.tile_pool(name="ps_q", bufs=2, space="PSUM"))
        ps_att = ctx.enter_context(tc.tile_pool(name="ps_att", bufs=2, space="PSUM"))
        ps_y = ctx.enter_context(tc.tile_pool(name="ps_y", bufs=2, space="PSUM"))

        # ---------- weight prep (sync-queue DMAs first, tiny) ----------
        w1_sb = singles.tile([CR, C], F32, tag="w1")  # [64, 512]
        nc.sync.dma_start(out=w1_sb, in_=w1_d)
        b1_sb = singles.tile([CR, 1], F32, tag="b1")
        nc.sync.dma_start(out=b1_sb, in_=b1_d.rearrange("(c one) -> c one", one=1))
        w2_sb = [small.tile([128, CR], F32, tag="w2c", name=f"w2c{oc}") for oc in range(KC)]
        for oc in range(KC):
            nc.sync.dma_start(out=w2_sb[oc], in_=w2_d[oc * 128 : (oc + 1) * 128, :])
        b2_stage = singles.tile([1, C], F32, tag="b2stage")
        nc.sync.dma_start(out=b2_stage, in_=b2_d.rearrange("(one c) -> one c", one=1))

        # ---------- all x loads (sync queue; stores will follow FIFO) ----------
        xts = []
        for s in range(BPC):
            xt = [
                singles.tile([128, N], BF16, tag=f"x{s}_{k}", name=f"x{s}_{k}")
                for k in range(KC)
            ]
            for piece in range(NL):
                lsl = bass.ts(piece, LF)
                for k in range(KC):
                    nc.sync.dma_start(out=xt[k][:, lsl], in_=x_d[s, k * 128 : (k + 1) * 128, lsl])
            xts.append(xt)

        # ---------- constants / transposed weights ----------
        ident = singles.tile([128, 128], BF16, tag="ident")
        make_identity(nc, ident)
        identf = singles.tile([128, 128], F32, tag="identf")
        make_identity(nc, identf)

        # w1T: [128, 4, 64] bf16 (chunk k = w1[:, 128k:128k+128].T)
        w1T = singles.tile([128, KC, CR], BF16, tag="w1T")
        for k in range(KC):
            ptp = ps_att.tile([128, CR], F32, tag="attp", name=f"w1tp{k}")
            nc.tensor.transpose(ptp, w1_sb[:, k * 128 : (k + 1) * 128], identf[0:CR, 0:CR])
            nc.vector.tensor_copy(w1T[:, k, :], ptp)

        # w2T: [64, 512] bf16, w2T[c, o] = w2[o, c]
        w2T = singles.tile([CR, C], BF16, tag="w2T")
        for oc in range(KC):
            ptp = ps_att.tile([CR, 128], F32, tag="attp", name=f"w2tp{oc}")
            nc.tensor.transpose(ptp, w2_sb[oc], identf)
            nc.vector.tensor_copy(w2T[:, oc * 128 : (oc + 1) * 128], ptp)

        # persistent per-sample tiles
        qas, qTs, GaTs = [], [], []
        for s in range(BPC):
            qa = singles.tile([CR + 1, N], BF16, tag=f"qa{s}")
            nc.gpsimd.memset(qa[CR : CR + 1, :], 1.0)
            qT = singles.tile([128, NT, CR], BF16, tag=f"qT{s}")
            GaT = singles.tile([CR + 1, C], BF16, tag=f"GaT{s}")
            nc.vector.tensor_copy(GaT[CR : CR + 1, :], b2_stage)
            qas.append(qa)
            qTs.append(qT)
            GaTs.append(GaT)

        # ---------- per-sample phases ----------
        def stream_block(s, n):
            nsl = bass.ts(n, NF)
            pq = ps_q.tile([CR, NF], F32, tag="mm", name=f"pq{s}_{n}")
            for k in range(KC):
                nc.tensor.matmul(
                    pq, w1T[:, k, :], xts[s][k][:, nsl],
                    start=(k == 0), stop=(k == KC - 1),
                )
            nc.scalar.activation(qas[s][0:CR, nsl], pq, AF.Identity, bias=b1_sb, scale=1.0)

        def transpose_q(s):
            # [64, 4096] -> [128, 32, 64] via the DMA XBAR (scalar HWDGE queue)
            nc.scalar.dma_start_transpose(qTs[s], qas[s][0:CR, :])

        def gram(s):
            patt = ps_att.tile([CR, CR], F32, tag="attp", name=f"att{s}")
            for t in range(NT):
                qTc = qTs[s][:, t, :]
                nc.tensor.matmul(patt, qTc, qTc, start=(t == 0), stop=(t == NT - 1))
            return patt

        def softmax_gt(s, patt):
            negm = small.tile([CR, 1], F32, tag="negm", name=f"negm{s}")
            nc.vector.tensor_reduce(out=negm, in_=patt, axis=AX.X, op=ALU.max, negate=True)
            shifted = small.tile([CR, CR], F32, tag="shifted", name=f"shifted{s}")
            nc.vector.tensor_scalar(
                out=shifted, in0=patt, scalar1=negm, scalar2=-80.0,
                op0=ALU.add, op1=ALU.max,
            )
            atte = small.tile([CR, CR], F32, tag="atte", name=f"atte{s}")
            ssum = small.tile([CR, 1], F32, tag="ssum", name=f"ssum{s}")
            nc.scalar.activation(atte, shifted, AF.Exp, bias=0.0, scale=1.0, accum_out=ssum)
            rsum = small.tile([CR, 1], F32, tag="rsum", name=f"rsum{s}")
            nc.vector.reciprocal(rsum, ssum)
            att = small.tile([CR, CR], BF16, tag="attn", name=f"attn{s}")
            nc.vector.tensor_scalar_mul(att, atte, rsum)
            # G^T[d, o] = sum_c att[c, d] * w2T[c, o]  (+ b2 in GaT row 64)
            pgt = ps_q.tile([CR, C], F32, tag="mm", name=f"pgt{s}")
            nc.tensor.matmul(pgt, att, w2T, start=True, stop=True)
            nc.scalar.copy(GaTs[s][0:CR, :], pgt)

        def y_unit(s, u):
            """One [128, EU] output unit: oc-chunk rows, EU-wide n-slice."""
            oc, iu = divmod(u, NU)
            osl = slice(oc * 128, (oc + 1) * 128)
            py = ps_y.tile([128, EU], F32, tag="y", name=f"py{s}_{u}")
            on_act = u in ACT_UNITS
            for sub in range(EU // NF):
                nsl = bass.ts(iu * (EU // NF) + sub, NF)
                psl = py[:, sub * NF : (sub + 1) * NF]
                if on_act:
                    nc.tensor.matmul(psl, ident, xts[s][oc][:, nsl], start=True, stop=False)
                    nc.tensor.matmul(psl, GaTs[s][:, osl], qas[s][:, nsl], start=False, stop=True)
                else:
                    nc.tensor.matmul(psl, GaTs[s][:, osl], qas[s][:, nsl], start=True, stop=True)
            f = fins[s][oc]
            fsl = f[:, iu * EU : (iu + 1) * EU]
            if on_act:
                nc.scalar.copy(fsl, py)
            else:
                nc.vector.tensor_add(fsl, py, xts[s][oc][:, iu * EU : (iu + 1) * EU])

        def store_oc(s, oc):
            osl = slice(oc * 128, (oc + 1) * 128)
            nc.sync.dma_start(out=out_d[s, osl, :], in_=fins[s][oc])

        fins = [
            [fin.tile([128, N], BF16, tag="fin", name=f"fin{s}_{oc}") for oc in range(KC)]
            for s in range(BPC)
        ]

        # sample 0 stream + gram + softmax; s1 stream blocks fill the PE
        # while s0's softmax chain runs on DVE/ACT.
        for n in range(NN):
            stream_block(0, n)
        transpose_q(0)
        patt0 = gram(0)
        stream_block(1, 0)
        softmax_gt(0, patt0)
        stream_block(1, 1)
        for i in range(NN):
            y_unit(0, 2 * i)
            y_unit(0, 2 * i + 1)
            if (2 * i + 1) % NU == NU - 1:
                store_oc(0, (2 * i + 1) // NU)
            if i + 2 < NN:
                stream_block(1, i + 2)
        transpose_q(1)
        patt1 = gram(1)
        softmax_gt(1, patt1)
        for u in range(4 * NU):
            y_unit(1, u)
            if u % NU == NU - 1:
                store_oc(1, u // NU)

    nc.compile()
    return nc


_NC_CACHE = None


def _get_nc():
    global _NC_CACHE
    if _NC_CACHE is None:
        _NC_CACHE = _build_nc()
    return _NC_CACHE


def _as_f32(a):
    return np.ascontiguousarray(np.asarray(a, dtype=np.float32))


def run(inputs, trace=False):
    """Run on all 8 cores; returns (full output [B,C,W,H], BassKernelResults)."""
    nc = _get_nc()
    x = np.ascontiguousarray(
        np.asarray(inputs["x"]).reshape(B, C, N).astype(ml_dtypes.bfloat16)
    )
    w1 = _as_f32(inputs["w1"])
    b1 = _as_f32(inputs["b1"])
    w2 = _as_f32(inputs["w2"])
    b2 = _as_f32(inputs["b2"])
    in_maps = [
        {
            "x": x[c * BPC : (c + 1) * BPC],
            "w1": w1,
            "b1": b1,
            "w2": w2,
            "b2": b2,
        }
        for c in range(NCORES)
    ]
    res = run_bass_kernel_spmd(nc, in_maps, list(range(NCORES)), trace=trace)
    out = np.concatenate([res.results[c]["out"] for c in range(NCORES)], axis=0)
    return out.reshape(B, C, W, H).astype(np.float32), res


def kernel(**inputs):
    out, _ = run(inputs)
    return out
